# revision 1
# baseline (speedup 1.0000x reference)
"""Trainium2 Bass kernel: single-head causal self-attention.

Reference computation (per batch item, B=512, T=128, C=384, H=64, fp32):
    q = x @ Wq; k = x @ Wk; v = x @ Wv          # [T, H]
    scores = (q @ k^T) * 6**-0.5                # [T, T], causal masked
    out = softmax(scores, axis=-1) @ v          # [T, H]

Sharding: pure data parallel, batch dim (512) split across 8 NeuronCores
(64 items per core); Wq/Wk/Wv replicated.

Per-core kernel plan (T=128 tokens sit on the 128 SBUF partitions):
  * x_b^T (chunks of C) via PE transposes — needed since all projections
    contract over C, which must live on the partition dim.
  * qT/kT = Wq^T x^T and Wk^T x^T, batched 4 items per matmul (N=512).
  * v = x_b @ Wv with xT chunks stationary.
  * scoresT[s,t] = sum_h kT[h,s] qT[h,t]; causal mask added inside the same
    PSUM accumulation group via psum += I^T @ maskM (maskM[s,t] = -1e10
    where s > t).
  * expm = exp(SCALE * scoresT) on the scalar engine (masked entries -> 0,
    so no max-subtraction needed; values are small enough for fp32).
  * o[t, 0:65] = expm^T @ [v | 1] — attention numerator plus row sums in
    one matmul (shared stationary).
  * y = o[:, 0:64] * (1 / o[:, 64]) per partition.
"""

import numpy as np

import concourse.bacc as bacc
import concourse.bass as bass
import concourse.mybir as mybir
import concourse.tile as tile
from concourse import bass_utils

B, T, C, H = 512, 128, 384, 64
N_CORES = 8
BPC = B // N_CORES  # batch items per core
GROUP = 4  # batch items per projection matmul (N = GROUP*T = 512)
NCHUNK = C // 128  # 3 chunks of the C (contraction) dim
SCALE = float(6 ** -0.5)
NEG = -1.0e10
F32 = mybir.dt.float32


def build_program(n_batch: int = BPC):
    """Build the per-core Bass program. Returns the compiled Bacc object."""
    nc = bacc.Bacc(
        "TRN2",
        target_bir_lowering=False,
        debug=False,
        enable_asserts=False,
        num_devices=N_CORES,
    )

    x_d = nc.dram_tensor("x", [n_batch, T, C], F32, kind="ExternalInput")
    wq_d = nc.dram_tensor("wq", [128, NCHUNK, H], F32, kind="ExternalInput")
    wk_d = nc.dram_tensor("wk", [128, NCHUNK, H], F32, kind="ExternalInput")
    wv_d = nc.dram_tensor("wv", [128, NCHUNK, H], F32, kind="ExternalInput")
    ident_d = nc.dram_tensor("ident", [128, 128], F32, kind="ExternalInput")
    maskm_d = nc.dram_tensor("maskm", [128, 128], F32, kind="ExternalInput")
    y_d = nc.dram_tensor("y", [n_batch, T, H], F32, kind="ExternalOutput")

    n_groups = n_batch // GROUP

    with tile.TileContext(nc) as tc:
        with (
            tc.tile_pool(name="const", bufs=1) as constp,
            tc.tile_pool(name="xin", bufs=2) as xinp,
            tc.tile_pool(name="xtp", bufs=2) as xtp,
            tc.tile_pool(name="qkp", bufs=2) as qkp,
            tc.tile_pool(name="work", bufs=3) as workp,
            tc.tile_pool(name="yout", bufs=2) as youtp,
            tc.tile_pool(name="pxt", bufs=2, space="PSUM") as pxt,
            tc.tile_pool(name="pqk", bufs=1, space="PSUM") as pqk,
            tc.tile_pool(name="psc", bufs=2, space="PSUM") as psc,
            tc.tile_pool(name="pout", bufs=2, space="PSUM") as pout,
        ):
            ident = constp.tile([128, 128], F32, name="ident_sb")
            maskm = constp.tile([128, 128], F32, name="maskm_sb")
            wq = constp.tile([128, NCHUNK, H], F32, name="wq_sb")
            wk = constp.tile([128, NCHUNK, H], F32, name="wk_sb")
            wv = constp.tile([128, NCHUNK, H], F32, name="wv_sb")
            zbias = constp.tile([128, 1], F32, name="zbias_sb")
            nc.sync.dma_start(ident[:], ident_d[:])
            nc.sync.dma_start(maskm[:], maskm_d[:])
            nc.sync.dma_start(wq[:], wq_d[:])
            nc.sync.dma_start(wk[:], wk_d[:])
            nc.sync.dma_start(wv[:], wv_d[:])
            nc.vector.memset(zbias[:], 0.0)

            for g in range(n_groups):
                bs = g * GROUP
                # ---- load x for the group: [T, GROUP, C] ----
                x_sb = xinp.tile([T, GROUP, C], F32, name="x_sb", tag="x_sb")
                nc.sync.dma_start(
                    x_sb[:], x_d[bs : bs + GROUP].rearrange("b t c -> t b c")
                )

                # ---- transpose x per item -> xt_sb [128(c), NCHUNK, GROUP, T] ----
                xt_sb = xtp.tile(
                    [128, NCHUNK, GROUP, T], F32, name="xt_sb", tag="xt_sb"
                )
                for b in range(GROUP):
                    xt_ps = pxt.tile([128, C], F32, name="xt_ps", tag="xt_ps")
                    for c in range(NCHUNK):
                        nc.tensor.transpose(
                            xt_ps[:, c * 128 : (c + 1) * 128],
                            x_sb[:, b, c * 128 : (c + 1) * 128],
                            ident[:],
                        )
                    nc.scalar.copy(
                        xt_sb[:, :, b, :],
                        xt_ps[:].rearrange("p (k t) -> p k t", k=NCHUNK),
                    )

                # ---- qT / kT, batched over the group: [H, GROUP*T] ----
                q_ps = pqk.tile([H, GROUP * T], F32, name="q_ps", tag="q_ps")
                k_ps = pqk.tile([H, GROUP * T], F32, name="k_ps", tag="k_ps")
                for c in range(NCHUNK):
                    nc.tensor.matmul(
                        q_ps[:],
                        wq[:, c, :],
                        xt_sb[:, c],
                        start=(c == 0),
                        stop=(c == NCHUNK - 1),
                    )
                for c in range(NCHUNK):
                    nc.tensor.matmul(
                        k_ps[:],
                        wk[:, c, :],
                        xt_sb[:, c],
                        start=(c == 0),
                        stop=(c == NCHUNK - 1),
                    )
                q_sb = qkp.tile([H, GROUP * T], F32, name="q_sb", tag="q_sb")
                k_sb = qkp.tile([H, GROUP * T], F32, name="k_sb", tag="k_sb")
                nc.vector.tensor_copy(q_sb[:], q_ps[:])
                nc.scalar.copy(k_sb[:], k_ps[:])

                y_sb = youtp.tile([T, GROUP, H], F32, name="y_sb", tag="y_sb")

                for b in range(GROUP):
                    # ---- v_b = x_b @ Wv : [T, H], accumulated over chunks ----
                    o_ps = pout.tile([T, 192], F32, name="o_ps", tag="o_ps")
                    for c in range(NCHUNK):
                        nc.tensor.matmul(
                            o_ps[:, 0:H],
                            xt_sb[:, c, b, :],
                            wv[:, c, :],
                            start=(c == 0),
                            stop=(c == NCHUNK - 1),
                        )
                    vext = workp.tile([T, H + 1], F32, name="vext", tag="vext")
                    nc.vector.memset(vext[:, H : H + 1], 1.0)
                    nc.vector.tensor_copy(vext[:, 0:H], o_ps[:, 0:H])

                    # ---- scoresT + causal mask in one PSUM group ----
                    sc_ps = psc.tile([T, T], F32, name="sc_ps", tag="sc_ps")
                    nc.tensor.matmul(
                        sc_ps[:],
                        k_sb[:, b * T : (b + 1) * T],
                        q_sb[:, b * T : (b + 1) * T],
                        start=True,
                        stop=False,
                    )
                    nc.tensor.matmul(
                        sc_ps[:], ident[:], maskm[:], start=False, stop=True
                    )

                    # ---- expm = exp(SCALE * scoresT) ----
                    expm = workp.tile([T, T], F32, name="expm", tag="expm")
                    nc.scalar.activation(
                        expm[:],
                        sc_ps[:],
                        mybir.ActivationFunctionType.Exp,
                        bias=zbias[:],
                        scale=SCALE,
                    )

                    # ---- numerator + row sums: expm^T @ [v | 1] ----
                    nc.tensor.matmul(
                        o_ps[:, 64 : 64 + H + 1],
                        expm[:],
                        vext[:],
                        start=True,
                        stop=True,
                    )
                    rcp = workp.tile([T, 1], F32, name="rcp", tag="rcp")
                    nc.vector.reciprocal(rcp[:], o_ps[:, 64 + H : 64 + H + 1])
                    nc.vector.tensor_scalar_mul(
                        y_sb[:, b, :], o_ps[:, 64 : 64 + H], rcp[:]
                    )

                # ---- store group output ----
                nc.sync.dma_start(
                    y_d[bs : bs + GROUP].rearrange("b t h -> t b h"), y_sb[:]
                )

    nc.compile()
    return nc


_CACHED = {}


def _get_program(n_batch: int = BPC):
    if n_batch not in _CACHED:
        _CACHED[n_batch] = build_program(n_batch)
    return _CACHED[n_batch]


def make_const_inputs():
    ident = np.eye(128, dtype=np.float32)
    s_idx = np.arange(128)[:, None]
    t_idx = np.arange(128)[None, :]
    maskm = np.where(s_idx > t_idx, np.float32(NEG), np.float32(0.0))
    return ident, maskm.astype(np.float32)


def prep_weight(w: np.ndarray) -> np.ndarray:
    # [C, H] -> [128(c within chunk), NCHUNK, H]
    return np.ascontiguousarray(
        w.reshape(NCHUNK, 128, H).transpose(1, 0, 2)
    ).astype(np.float32)


def kernel(x, Wq, Wk, Wv):
    x = np.ascontiguousarray(np.asarray(x), dtype=np.float32)
    wq = prep_weight(np.asarray(Wq))
    wk = prep_weight(np.asarray(Wk))
    wv = prep_weight(np.asarray(Wv))
    ident, maskm = make_const_inputs()

    nc = _get_program(BPC)

    in_maps = []
    for core in range(N_CORES):
        shard = np.ascontiguousarray(x[core * BPC : (core + 1) * BPC])
        in_maps.append(
            {
                "x": shard,
                "wq": wq,
                "wk": wk,
                "wv": wv,
                "ident": ident,
                "maskm": maskm,
            }
        )

    res = bass_utils.run_bass_kernel_spmd(nc, in_maps, core_ids=list(range(N_CORES)))
    out = np.concatenate([res.results[core]["y"] for core in range(N_CORES)], axis=0)
    return out.astype(np.float32)
